# revision 24
# baseline (speedup 1.0000x reference)
"""Trainium2 Bass kernel for CustomTaylorLayer.

Math: out[b,o] = sum_i weights[o,i] * P_{o,i}(x[b,i]) with
P_{o,i}(t) = sum_{k=0..5} coeffs[o,i,k] t^k  (Horner in the reference).

Expanded:  out[b,o] = sum_{k,i} (coeffs[o,i,k]*weights[o,i]) * x[b,i]^k
i.e. a single matmul  out = Xpow @ Wcat  with
  Xpow = [1 | x | x^2 | ... | x^5]  : [B, 6*I]
  Wcat[k*I+i, o] = coeffs[o,i,k] * weights[o,i]

Sharding: data-parallel over batch: 8 cores x B_loc=256. coeffs/weights
replicated (small). Per core: 12 accumulating matmuls of [128K,128M]@[128K,256N]
x 2 output row-blocks, fp32r (single-pass fp32 streaming on the PE).
"""

import numpy as np

B, O, I, NK = 2048, 256, 256, 6
NCORES = 8
BLOC = B // NCORES  # 256

_cache = {}


def _build(mm_dtype_name="float32r", tr_dtype_name="float32", loop_iters=None, loop_mode="full", coalesce=False, dual_ring=False, swdge_coeffs=False, merge_k=True, interleave_mm=True):
    import contextlib
    import concourse.mybir as mybir
    from concourse import bacc
    from concourse.tile import TileContext
    from concourse.masks import make_identity

    F32 = mybir.dt.float32
    MMDT = getattr(mybir.dt, mm_dtype_name)
    TRDT = getattr(mybir.dt, tr_dtype_name)
    Square = mybir.ActivationFunctionType.Square

    def _ap(t):
        import concourse.bass as _bass
        return t if isinstance(t, _bass.AP) else t[:]

    nc = bacc.Bacc("TRN2", target_bir_lowering=False, debug=False)
    x_d = nc.dram_tensor("x", [BLOC, I], F32, kind="ExternalInput")
    c_d = nc.dram_tensor("coeffs", [O, I * NK], F32, kind="ExternalInput")
    w_d = nc.dram_tensor("weights", [O, I], F32, kind="ExternalInput")
    o_d = nc.dram_tensor("out", [BLOC, O], F32, kind="ExternalOutput")

    with TileContext(nc) as tc:
        with (
            tc.tile_pool(name="sb", bufs=1) as pool,
            tc.tile_pool(name="tp", bufs=6, space="PSUM") as pst,
            tc.tile_pool(name="pacc", bufs=1, space="PSUM") as pacc,
        ):
            def emit_loads():
                ring2 = nc.scalar if dual_ring else nc.sync
                if coalesce:
                    # one DMA per logical tensor: fold the row-block pair into
                    # the free dim ([256,N] dram -> [128, 2, N] sbuf)
                    xt = pool.tile([128, 2, I], F32, tag="x", name="xt")
                    ring2.dma_start(xt[:], x_d.ap().rearrange("(two p) i -> p two i", p=128))
                    xs = [xt[:, bh, :] for bh in range(2)]
                    wt = pool.tile([128, 2, I], F32, tag="w", name="wt")
                    ring2.dma_start(wt[:], w_d.ap().rearrange("(two p) i -> p two i", p=128))
                    ws = [wt[:, oh, :] for oh in range(2)]
                    cs4 = {}
                    cring = nc.gpsimd if swdge_coeffs else nc.sync
                    for ih in range(2):
                        t = pool.tile([128, 2, NK * 128], F32, tag=f"c{ih}", name=f"c{ih}")
                        cring.dma_start(
                            t[:],
                            c_d.ap().rearrange("(two p) f -> p two f", p=128)[
                                :, :, ih * NK * 128 : (ih + 1) * NK * 128
                            ],
                        )
                        for oh in range(2):
                            cs4[(oh, ih)] = t[:, oh, :]
                    return xs, ws, cs4
                xs = []
                for bh in range(2):
                    t = pool.tile([128, I], F32, tag=f"x{bh}", name=f"x{bh}")
                    ring2.dma_start(t[:], x_d.ap()[bh * 128 : (bh + 1) * 128, :])
                    xs.append(t)
                ws = []
                for oh in range(2):
                    t = pool.tile([128, I], F32, tag=f"w{oh}", name=f"w{oh}")
                    ring2.dma_start(t[:], w_d.ap()[oh * 128 : (oh + 1) * 128, :])
                    ws.append(t)
                cs4 = {}
                cring = nc.gpsimd if swdge_coeffs else nc.sync
                for ih in range(2):
                    for oh in range(2):
                        t = pool.tile([128, NK * 128], F32, tag=f"c{oh}_{ih}", name=f"c{oh}_{ih}")
                        cring.dma_start(
                            t[:],
                            c_d.ap()[
                                oh * 128 : (oh + 1) * 128,
                                ih * NK * 128 : (ih + 1) * NK * 128,
                            ],
                        )
                        cs4[(oh, ih)] = t
                return xs, ws, cs4

            def emit_compute(xs, ws, cs4):
                # warm the ACT function table at t=0 (no input deps) so later
                # activation ops don't pay the ~1.3us table load mid-cascade
                warm = pool.tile([128, 1], F32, tag="warm", name="warm")
                nc.scalar.activation(
                    warm[:], nc.const_aps.aps[(F32, 1.0)],
                    mybir.ActivationFunctionType.Square,
                )
                ident_f = pool.tile([128, 128], F32, tag="ident_f", name="ident_f")
                make_identity(nc, ident_f[:])
                if TRDT != F32:
                    # transposes in fp32r need an fp32r-rounded identity; the
                    # gpsimd affine_select can only write f32, so round via ACT
                    ident = pool.tile([128, 128], TRDT, tag="ident", name="ident")
                    nc.scalar.copy(ident[:], ident_f[:])
                else:
                    ident = ident_f
                ones = pool.tile([128, BLOC], MMDT, tag="ones", name="ones")
                # ISA Memset can't write fp32r; ACT copy with scale=0, bias=1
                nc.scalar.activation(
                    ones[:], _ap(xs[0]), mybir.ActivationFunctionType.Copy,
                    bias=1.0, scale=0.0,
                )

                # transpose x -> p1 holds x^T for both i-chunks:
                # p1[:, ih*BLOC + b] with partitions = i (within chunk ih)
                p1 = pool.tile([128, 2 * BLOC], MMDT, tag="p1", name="p1")
                for ih in range(2):
                    ps = pst.tile([128, 2 * O], F32, tag="tp", name="tp_x")
                    for bh in range(2):
                        nc.tensor.transpose(
                            ps[:, bh * 128 : (bh + 1) * 128].bitcast(TRDT),
                            _ap(xs[bh])[:, ih * 128 : (ih + 1) * 128].bitcast(TRDT),
                            ident[:] if TRDT != F32 else ident[:].bitcast(TRDT),
                        )
                    nc.scalar.copy(p1[:, ih * BLOC : (ih + 1) * BLOC], ps[:, :BLOC])

                # transpose weights -> wT[ih] [128(i), O(o)]
                wT = []
                for ih in range(2):
                    ps = pst.tile([128, 2 * O], F32, tag="tp", name="tp_w")
                    for oh in range(2):
                        nc.tensor.transpose(
                            ps[:, oh * 128 : (oh + 1) * 128].bitcast(TRDT),
                            _ap(ws[oh])[:, ih * 128 : (ih + 1) * 128].bitcast(TRDT),
                            ident[:] if TRDT != F32 else ident[:].bitcast(TRDT),
                        )
                    t = pool.tile([128, O], F32, tag=f"wT{ih}", name=f"wT{ih}")
                    nc.scalar.copy(t[:], ps[:, :O])
                    wT.append(t)

                # powers of x^T, one [128, 512] op per power level
                def newp(k):
                    return pool.tile([128, 2 * BLOC], MMDT, tag=f"p{k}", name=f"p{k}")

                p2, p3, p4, p5 = newp(2), newp(3), newp(4), newp(5)
                nc.scalar.activation(p2[:], p1[:].bitcast(F32), Square)
                nc.vector.tensor_mul(p3[:], p2[:].bitcast(F32), p1[:].bitcast(F32))
                nc.scalar.activation(p4[:], p2[:].bitcast(F32), Square)
                nc.vector.tensor_mul(p5[:], p4[:].bitcast(F32), p1[:].bitcast(F32))
                powers = {1: p1, 2: p2, 3: p3, 4: p4, 5: p5}

                def lhsT(k, ih, bh):
                    if k == 0:
                        return ones[:, bh * 128 : (bh + 1) * 128]
                    return powers[k][:, ih * BLOC + bh * 128 : ih * BLOC + (bh + 1) * 128]

                # coeffs transpose + fused product with wT
                wk = {}
                if merge_k:
                    # two k-planes share one [128,512] PSUM bank, one big mul
                    for ih in range(2):
                        for kp in range(NK // 2):
                            ps = pst.tile([128, 2 * O], F32, tag="tp", name="tp_o")
                            for kk in range(2):
                                k = 2 * kp + kk
                                for oh in range(2):
                                    src = _ap(cs4[(oh, ih)]).rearrange(
                                        "p (i s) -> p i s", s=NK
                                    )[:, :, k]
                                    nc.tensor.transpose(
                                        ps[:, kk * O + oh * 128 : kk * O + (oh + 1) * 128].bitcast(TRDT),
                                        src.bitcast(TRDT),
                                        ident[:] if TRDT != F32 else ident[:].bitcast(TRDT),
                                    )
                            t = pool.tile([128, 2 * O], MMDT, tag=f"wk{kp}_{ih}", name=f"wk{kp}_{ih}")
                            wtb = wT[ih][:].unsqueeze(1).broadcast_to([128, 2, O])
                            nc.vector.tensor_mul(
                                t[:].rearrange("p (two o) -> p two o", two=2), ps[:].rearrange("p (two o) -> p two o", two=2), wtb
                            )
                            for kk in range(2):
                                wk[(2 * kp + kk, ih)] = t[:, kk * O : (kk + 1) * O]
                else:
                    for ih in range(2):
                        for k in range(NK):
                            ps = pst.tile([128, 2 * O], F32, tag="tp", name="tp_o")
                            for oh in range(2):
                                src = _ap(cs4[(oh, ih)]).rearrange(
                                    "p (i s) -> p i s", s=NK
                                )[:, :, k]
                                nc.tensor.transpose(
                                    ps[:, oh * 128 : (oh + 1) * 128].bitcast(TRDT),
                                    src.bitcast(TRDT),
                                    ident[:] if TRDT != F32 else ident[:].bitcast(TRDT),
                                )
                            t = pool.tile([128, O], MMDT, tag=f"wk1{k}_{ih}", name=f"wk1{k}_{ih}")
                            nc.vector.tensor_mul(t[:], ps[:, :O], wT[ih][:])
                            wk[(k, ih)] = t[:]

                # the 2x12 accumulating matmuls, interleaved per chunk so the
                # last-arriving chunk finishes both groups back-to-back; both
                # accumulators share one full PSUM bank -> one copy + one DMA
                chunks = [(k, ih) for ih in range(2) for k in range(NK)]
                accs = [
                    pacc.tile([128, O], F32, tag=f"acc{bh}", name=f"acc{bh}")
                    for bh in range(2)
                ]
                if interleave_mm:
                    for j, (k, ih) in enumerate(chunks):
                        for bh in range(2):
                            nc.tensor.matmul(
                                accs[bh][:],
                                lhsT(k, ih, bh),
                                wk[(k, ih)],
                                start=(j == 0),
                                stop=(j == len(chunks) - 1),
                            )
                else:
                    for bh in range(2):
                        for j, (k, ih) in enumerate(chunks):
                            nc.tensor.matmul(
                                accs[bh][:],
                                lhsT(k, ih, bh),
                                wk[(k, ih)],
                                start=(j == 0),
                                stop=(j == len(chunks) - 1),
                            )
                osb = pool.tile([128, 2 * O], F32, tag="osb", name="osb")
                for bh in range(2):
                    nc.vector.tensor_copy(osb[:, bh * O : (bh + 1) * O], accs[bh][:])
                nc.sync.dma_start(
                    o_d.ap().rearrange("(two p) o -> p two o", p=128),
                    osb[:].rearrange("p (two o) -> p two o", two=2)
                )

            if loop_iters is None:
                emit_compute(*emit_loads())
            elif loop_mode == "full":
                with tc.For_i(0, loop_iters, 1):
                    emit_compute(*emit_loads())
            elif loop_mode == "compute":
                tiles = emit_loads()
                with tc.For_i(0, loop_iters, 1):
                    emit_compute(*tiles)
            elif loop_mode == "dma":
                with tc.For_i(0, loop_iters, 1):
                    emit_loads()
                emit_compute(*emit_loads())
            else:
                raise ValueError(loop_mode)

    nc.compile()
    return nc


def _get_nc(key="default"):
    if key not in _cache:
        _cache[key] = _build()
    return _cache[key]


def kernel(x, coeffs, weights):
    from concourse.bass_utils import run_bass_kernel_spmd

    x = np.ascontiguousarray(np.asarray(x, dtype=np.float32))
    cflat = np.ascontiguousarray(
        np.asarray(coeffs, dtype=np.float32).reshape(O, I * NK)
    )
    w = np.ascontiguousarray(np.asarray(weights, dtype=np.float32))

    nc = _get_nc()
    in_maps = [
        {"x": x[c * BLOC : (c + 1) * BLOC], "coeffs": cflat, "weights": w}
        for c in range(NCORES)
    ]
    res = run_bass_kernel_spmd(nc, in_maps, core_ids=list(range(NCORES)))
    out = np.concatenate([r["out"] for r in res.results], axis=0)
    return out
